# Initial kernel scaffold
#
"""Trainium2 Bass kernel for the SSIM+KLDiv nn_KLD problem.

Contract: kernel(**inputs) takes FULL unsharded inputs (img1, img2, window:
numpy arrays) and returns the FULL output (scalar float32), distributing work
across 8 NeuronCores internally.

Math (matching reference.py):
  mu1 = conv(img1), mu2 = conv(img2)  [depthwise 11x11 gaussian, 'same' pad]
  sigma terms from conv(img1^2), conv(img2^2), conv(img1*img2)
  ssim = mean of per-pixel SSIM map
  kl from per-row histograms -> softmax -> KLDiv (only used if ssim > 0.75)
  out = kl + 1 - ssim if ssim > 0.75 else 1 - ssim

Device strategy (per core, 32 image pairs):
  planes s=x+y, d=x-y, s^2, d^2 (bf16)
  H-conv on PE with the plane as the stationary operand -> transposed
  intermediate in PSUM; W-conv on PE with shared banded stationary.
  Conv linearity gives mu1+-mu2 = conv(s|d), conv(xy)=(conv(s^2)-conv(d^2))/4,
  conv(x^2+y^2)=(conv(s^2)+conv(d^2))/2.
  Pointwise SSIM on ACT (PSUM evac fused with Square/scale) + DVE
  (affine_then_add chains, scalar_tensor_tensor with fused accum reduction).
  Host: sum partials across cores, final scalar combine.
"""

import sys

sys.path.insert(0, "/opt/trn_rl_repo")

import math

import numpy as np

import concourse.bass as bass  # noqa: F401  (bass types used via bacc/tile)
import concourse.tile as tile
from concourse import bacc, mybir
from concourse.bass_utils import run_bass_kernel_spmd

# Problem constants (hardcoded per the harness contract).
B, C, H, W = 256, 1, 192, 256
NCORES = 8
PPC = B // NCORES  # image pairs per core
WS = 11
SIGMA = 1.5
NBIN = 1000
C1 = 0.01**2
C2 = 0.03**2
HHI, HLO = 128, H - 128  # h-partition split (128 + 64)
WHALF = 128  # w split (2 x 128)

F32 = mybir.dt.float32
BF16 = mybir.dt.bfloat16

_CACHE = {}


def _gauss_taps():
    g = np.array(
        [math.exp(-((i - WS // 2) ** 2) / (2.0 * SIGMA**2)) for i in range(WS)],
        dtype=np.float64,
    )
    g = g / g.sum()
    return g.astype(np.float32)


def _make_bands(g):
    """Banded 1-D conv matrices.

    A[h, h'] = g[h - h' + 5]   (H-conv: out[h'] = sum_h A[h,h'] x[h])
    Bm[w, w'] = g[w - w' + 5]  (W-conv)
    """
    A = np.zeros((H, H), dtype=np.float32)
    for h in range(H):
        for hp in range(max(0, h - 5), min(H, h + 6)):
            A[h, hp] = g[h - hp + 5]
    Bm = np.zeros((W, W), dtype=np.float32)
    for w in range(W):
        for wp in range(max(0, w - 5), min(W, w + 6)):
            Bm[w, wp] = g[w - wp + 5]
    bf = np.float32  # cast to bf16 happens via ml_dtypes below
    import ml_dtypes

    to_bf16 = lambda a: a.astype(ml_dtypes.bfloat16)
    return (
        to_bf16(A[0:HHI, :]),  # bandA_hi [128, 192]
        to_bf16(A[HHI:H, :]),  # bandA_lo [64, 192]
        to_bf16(Bm[0:WHALF, :]),  # bandB0  [128, 256]
        to_bf16(Bm[WHALF:W, :]),  # bandB1  [128, 256]
    )


def _build_nc():
    """Build + finalize the per-core Bass program (same program on all 8)."""
    nc = bacc.Bacc(None, target_bir_lowering=False, debug=False)

    x_in = nc.dram_tensor("img1", [PPC, H, W], F32, kind="ExternalInput")
    y_in = nc.dram_tensor("img2", [PPC, H, W], F32, kind="ExternalInput")
    bandA_hi = nc.dram_tensor("bandA_hi", [HHI, H], BF16, kind="ExternalInput")
    bandA_lo = nc.dram_tensor("bandA_lo", [HLO, H], BF16, kind="ExternalInput")
    bandB0 = nc.dram_tensor("bandB0", [WHALF, W], BF16, kind="ExternalInput")
    bandB1 = nc.dram_tensor("bandB1", [WHALF, W], BF16, kind="ExternalInput")
    partials_out = nc.dram_tensor("partials", [128, 1], F32, kind="ExternalOutput")

    SQH = math.sqrt(0.5)
    DG = 4  # pairs per DMA group

    with tile.TileContext(nc) as tc:
        with (
            tc.tile_pool(name="consts", bufs=1) as consts,
            tc.tile_pool(name="inp", bufs=3) as inp,
            tc.tile_pool(name="planes", bufs=3) as planes_pool,
            tc.tile_pool(name="zt", bufs=3) as zt_pool,
            tc.tile_pool(name="stg", bufs=3) as stg,
            tc.tile_pool(name="pw", bufs=2) as pw,
            tc.tile_pool(name="acc", bufs=1) as accp,
            tc.tile_pool(name="hpsum", bufs=2, space="PSUM") as hpsum,
            tc.tile_pool(name="wpsum", bufs=2, space="PSUM") as wpsum,
        ):
            # Constants into SBUF.
            A_hi = consts.tile([HHI, H], BF16)
            nc.gpsimd.dma_start(out=A_hi, in_=bandA_hi[:, :])
            A_lo = consts.tile([HLO, H], BF16)
            nc.gpsimd.dma_start(out=A_lo, in_=bandA_lo[:, :])
            B0 = consts.tile([WHALF, W], BF16)
            nc.gpsimd.dma_start(out=B0, in_=bandB0[:, :])
            B1 = consts.tile([WHALF, W], BF16)
            nc.gpsimd.dma_start(out=B1, in_=bandB1[:, :])
            # Rank-1 constant fold: ones[K=1,M=128] x crow[K=1,N=384] adds
            # per-column constants to a PSUM tile (2*(C1+C2) on the A2 half,
            # pre-ACT-scale 0.5).
            onesc = consts.tile([1, 128], BF16)
            nc.vector.memset(onesc, 1.0)
            crow = consts.tile([1, 2 * H], BF16)
            nc.vector.memset(crow[:, 0:H], 2.0 * (C1 + C2))
            nc.vector.memset(crow[:, H : 2 * H], 0.0)

            accT = accp.tile([128, 2 * H], BF16)
            nc.vector.memset(accT, 0.0)
            acc32 = accp.tile([128, 4], F32)
            nc.vector.memset(acc32, 0.0)

            def load_group(pg):
                p0 = pg * DG
                xgh = inp.tile([HHI, DG, W], BF16, tag="xgh", name="xgh")
                nc.gpsimd.dma_start(
                    out=xgh, in_=x_in[p0 : p0 + DG, 0:HHI, :].transpose([1, 0, 2])
                )
                xgl = inp.tile([HLO, DG, W], BF16, tag="xgl", name="xgl")
                nc.gpsimd.dma_start(
                    out=xgl, in_=x_in[p0 : p0 + DG, HHI:H, :].transpose([1, 0, 2])
                )
                ygh = inp.tile([HHI, DG, W], BF16, tag="ygh", name="ygh")
                nc.gpsimd.dma_start(
                    out=ygh, in_=y_in[p0 : p0 + DG, 0:HHI, :].transpose([1, 0, 2])
                )
                ygl = inp.tile([HLO, DG, W], BF16, tag="ygl", name="ygl")
                nc.gpsimd.dma_start(
                    out=ygl, in_=y_in[p0 : p0 + DG, HHI:H, :].transpose([1, 0, 2])
                )
                return xgh, xgl, ygh, ygl

            def stage2(grp, pj):
                """s,d,s2,d2: hi chunks on gpsimd, lo chunks on DVE."""
                xgh, xgl, ygh, ygl = grp
                xh, xl = xgh[:, pj, :], xgl[:, pj, :]
                yh, yl = ygh[:, pj, :], ygl[:, pj, :]
                sh = planes_pool.tile([HHI, W], BF16, tag="sh", name="sh")
                nc.gpsimd.tensor_add(sh, xh, yh)
                dh = planes_pool.tile([HHI, W], BF16, tag="dh", name="dh")
                nc.gpsimd.tensor_sub(dh, xh, yh)
                sl = planes_pool.tile([HLO, W], BF16, tag="sl", name="sl")
                nc.vector.tensor_add(sl, xl, yl)
                dl = planes_pool.tile([HLO, W], BF16, tag="dl", name="dl")
                nc.vector.tensor_sub(dl, xl, yl)
                s2h = planes_pool.tile([HHI, W], BF16, tag="s2h", name="s2h")
                nc.gpsimd.tensor_mul(s2h, sh, sh)
                d2h = planes_pool.tile([HHI, W], BF16, tag="d2h", name="d2h")
                nc.gpsimd.tensor_mul(d2h, dh, dh)
                s2l = planes_pool.tile([HLO, W], BF16, tag="s2l", name="s2l")
                nc.vector.tensor_mul(s2l, sl, sl)
                d2l = planes_pool.tile([HLO, W], BF16, tag="d2l", name="d2l")
                nc.vector.tensor_mul(d2l, dl, dl)
                return [((sh, sl), (dh, dl)), ((s2h, s2l), (d2h, d2l))]

            def hconv(units):
                hps = []
                for ui, (pa, pb) in enumerate(units):
                    hp = hpsum.tile([WHALF, 2, 512], F32, tag="hp", name="hp")
                    for qi, (qh, ql) in enumerate((pa, pb)):
                        for m in range(2):
                            dst = hp[:, qi, m * H : (m + 1) * H]
                            nc.tensor.matmul(
                                dst, qh[:, m * WHALF : (m + 1) * WHALF],
                                A_hi[:, :], start=True, stop=False,
                            )
                            nc.tensor.matmul(
                                dst, ql[:, m * WHALF : (m + 1) * WHALF],
                                A_lo[:, :], start=False, stop=True,
                            )
                    hps.append(hp)
                return hps

            def evac(hps):
                zsbs = []
                for ui, hp in enumerate(hps):
                    zsb = zt_pool.tile(
                        [WHALF, 2, 2 * H], BF16, tag=f"zsb{ui}", name="zsb"
                    )
                    nc.scalar.copy(out=zsb, in_=hp[:, :, 0 : 2 * H])
                    zsbs.append(zsb)
                return zsbs

            def wconv(zsbs):
                Ms = []
                for ui, zsb in enumerate(zsbs):
                    M = wpsum.tile([WHALF, 2, 512], F32, tag="M", name="M")
                    for m in range(2):
                        dst = M[:, m, 0 : 2 * H]
                        if m == 0:
                            nc.tensor.matmul(
                                dst, B0[:, 0:WHALF], zsb[:, :, 0:H],
                                start=True, stop=False,
                            )
                            nc.tensor.matmul(
                                dst, B1[0:5, 0:WHALF], zsb[0:5, :, H : 2 * H],
                                start=False, stop=(ui == 0),
                            )
                        else:
                            nc.tensor.matmul(
                                dst, B0[64:WHALF, WHALF:W],
                                zsb[64:WHALF, :, 0:H],
                                start=True, stop=False,
                            )
                            nc.tensor.matmul(
                                dst, B1[:, WHALF:W], zsb[:, :, H : 2 * H],
                                start=False, stop=(ui == 0),
                            )
                        if ui == 1:
                            # fold per-column constants into [A2|B2]
                            nc.tensor.matmul(
                                dst, onesc[:, :], crow[:, :],
                                start=False, stop=True,
                            )
                    Ms.append(M)
                return Ms

            def entry(Ms):
                ent = []
                for ui, M in enumerate(Ms):
                    t = stg.tile(
                        [WHALF, 2, 2 * H], BF16, tag=f"ent{ui}", name=f"ent{ui}"
                    )
                    if ui == 0:
                        nc.scalar.activation(
                            out=t, in_=M[:, :, 0 : 2 * H],
                            func=mybir.ActivationFunctionType.Square, scale=SQH,
                        )
                    else:
                        nc.scalar.activation(
                            out=t, in_=M[:, :, 0 : 2 * H],
                            func=mybir.ActivationFunctionType.Copy, scale=0.5,
                        )
                    ent.append(t)
                return ent

            def pointwise(ent):
                SQt, ABt = ent
                Ssq = SQt[:, :, 0:H]
                Qsq = SQt[:, :, H : 2 * H]
                As = ABt[:, :, 0:H]   # A2/2 + C1 + C2
                Bs = ABt[:, :, H : 2 * H]  # B2/2

                def pwt(tag, dt=BF16):
                    return pw.tile([WHALF, 2, H], dt, tag=tag, name=tag)

                F = pwt("F")
                nc.vector.tensor_sub(F, Ssq, Qsq)  # 2P
                G = pwt("G")
                nc.vector.tensor_add(G, Ssq, Qsq)  # d1
                Am = pwt("Am")
                nc.vector.tensor_sub(Am, As, Bs)  # 2D + C1 + C2
                Ap = pwt("Ap")
                nc.vector.tensor_add(Ap, As, Bs)  # U + C1 + C2
                num1 = pwt("num1")
                nc.vector.tensor_scalar(
                    out=num1, in0=F, scalar1=C1, scalar2=None,
                    op0=mybir.AluOpType.add,
                )
                den1 = pwt("den1")
                nc.vector.tensor_scalar(
                    out=den1, in0=G, scalar1=C1, scalar2=None,
                    op0=mybir.AluOpType.add,
                )
                num2 = pwt("num2")
                nc.vector.tensor_sub(num2, Am, num1)  # 2D + C2 - 2P
                den2 = pwt("den2")
                nc.vector.tensor_sub(den2, Ap, den1)  # U + C2 - d1
                num = pwt("num")
                nc.vector.tensor_mul(num, num1, num2)
                den = pwt("den", F32)
                nc.vector.tensor_mul(den, den1, den2)
                r = pwt("r", F32)
                nc.vector.reciprocal_approx_fast(out=r, in_=den)
                sp = pwt("sp")
                nc.vector.tensor_mul(sp, num, r)
                nc.vector.tensor_add(
                    accT, accT, sp.rearrange("p a h -> p (a h)")
                )

            # Software pipeline with per-tick emission order tuned for the
            # in-order engine streams (PE must never wait on the DVE tail).
            grp = load_group(0)
            units = stage2(grp, 0)
            hps = hconv(units)
            pending = evac(hps)
            for p in range(1, PPC + 1):
                if p < PPC:
                    if p % DG == 0:
                        grp = load_group(p // DG)
                    units = stage2(grp, p % DG)
                    hps = hconv(units)
                Ms = wconv(pending)
                ent = entry(Ms)
                if p < PPC:
                    pending = evac(hps)
                pointwise(ent)

            # ---- final reduction: bf16 accT -> f32 scalar-ish ----
            nc.vector.tensor_reduce(
                acc32[:, 0:1], accT, axis=mybir.AxisListType.X,
                op=mybir.AluOpType.add,
            )
            nc.gpsimd.dma_start(out=partials_out[:, :], in_=acc32[:, 0:1])

    nc.finalize()
    return nc


def _get_nc():
    if "nc" not in _CACHE:
        _CACHE["nc"] = _build_nc()
    return _CACHE["nc"]


def _host_kl(img1, img2):
    """Host-side KLDiv branch value (only consumed when ssim > 0.75)."""
    x1 = img1.reshape(B, H * W).astype(np.float32)
    x2 = img2.reshape(B, H * W).astype(np.float32)

    def row_hist(x):
        mn = x.min(axis=1, keepdims=True)
        mx = x.max(axis=1, keepdims=True)
        width = mx - mn
        scaled = np.where(width > 0, (x - mn) * NBIN / width, 0.0)
        idx = np.clip(scaled.astype(np.int32), 0, NBIN - 1)
        h = np.zeros((B, NBIN), np.float32)
        for r in range(B):
            h[r] = np.bincount(idx[r], minlength=NBIN)
        return h

    def softmax(h):
        e = np.exp(h - h.max(axis=1, keepdims=True))
        return e / e.sum(axis=1, keepdims=True)

    p1 = softmax(row_hist(x1))
    p2 = softmax(row_hist(x2))
    return float(np.sum(np.exp(p2) * (p2 - p1)) / B)


def kernel(img1, img2, window):
    img1 = np.asarray(img1, dtype=np.float32)
    img2 = np.asarray(img2, dtype=np.float32)
    window = np.asarray(window, dtype=np.float32)

    # Recover the 1-D taps from the passed 2-D window (rows sum to g_i since
    # sum(g)=1), keeping the kernel faithful to the provided window input.
    g = window[0, 0].sum(axis=1)
    g = (g / g.sum()).astype(np.float32)
    bandA_hi, bandA_lo, bandB0, bandB1 = _make_bands(g)

    x = img1.reshape(B, H, W)
    y = img2.reshape(B, H, W)

    nc = _get_nc()
    in_maps = []
    for c in range(NCORES):
        sl = slice(c * PPC, (c + 1) * PPC)
        in_maps.append(
            {
                "img1": np.ascontiguousarray(x[sl]),
                "img2": np.ascontiguousarray(y[sl]),
                "bandA_hi": bandA_hi,
                "bandA_lo": bandA_lo,
                "bandB0": bandB0,
                "bandB1": bandB1,
            }
        )

    res = run_bass_kernel_spmd(nc, in_maps, core_ids=list(range(NCORES)))
    total = 0.0
    for c in range(NCORES):
        total += float(res.results[c]["partials"].sum())
    ssim = total / float(B * C * H * W)

    if ssim > 0.75:
        out = _host_kl(img1, img2) + 1.0 - ssim
    else:
        out = 1.0 - ssim
    return np.float32(out)


if __name__ == "__main__":
    rng = np.random.default_rng(0)
    i1 = rng.standard_normal((B, C, H, W), dtype=np.float32)
    i2 = rng.standard_normal((B, C, H, W), dtype=np.float32)
    g = _gauss_taps()
    w2 = np.outer(g, g).astype(np.float32)[None, None]
    print("out:", kernel(i1, i2, w2))



# revision 1
# speedup vs baseline: 1.0027x; 1.0027x over previous
"""Trainium2 Bass kernel for the SSIM+KLDiv nn_KLD problem.

Contract: kernel(**inputs) takes FULL unsharded inputs (img1, img2, window:
numpy arrays) and returns the FULL output (scalar float32), distributing work
across 8 NeuronCores internally.

Math (matching reference.py):
  mu1 = conv(img1), mu2 = conv(img2)  [depthwise 11x11 gaussian, 'same' pad]
  sigma terms from conv(img1^2), conv(img2^2), conv(img1*img2)
  ssim = mean of per-pixel SSIM map
  kl from per-row histograms -> softmax -> KLDiv (only used if ssim > 0.75)
  out = kl + 1 - ssim if ssim > 0.75 else 1 - ssim

Device strategy (per core, 32 image pairs):
  planes s=x+y, d=x-y, s^2, d^2 (bf16)
  H-conv on PE with the plane as the stationary operand -> transposed
  intermediate in PSUM; W-conv on PE with shared banded stationary.
  Conv linearity gives mu1+-mu2 = conv(s|d), conv(xy)=(conv(s^2)-conv(d^2))/4,
  conv(x^2+y^2)=(conv(s^2)+conv(d^2))/2.
  Pointwise SSIM on ACT (PSUM evac fused with Square/scale) + DVE
  (affine_then_add chains, scalar_tensor_tensor with fused accum reduction).
  Host: sum partials across cores, final scalar combine.
"""

import sys

sys.path.insert(0, "/opt/trn_rl_repo")

import math

import numpy as np

import concourse.bass as bass  # noqa: F401  (bass types used via bacc/tile)
import concourse.tile as tile
from concourse import bacc, mybir
from concourse.bass_utils import run_bass_kernel_spmd

# Problem constants (hardcoded per the harness contract).
B, C, H, W = 256, 1, 192, 256
NCORES = 8
PPC = B // NCORES  # image pairs per core
WS = 11
SIGMA = 1.5
NBIN = 1000
C1 = 0.01**2
C2 = 0.03**2
HHI, HLO = 128, H - 128  # h-partition split (128 + 64)
WHALF = 128  # w split (2 x 128)

F32 = mybir.dt.float32
BF16 = mybir.dt.bfloat16

_CACHE = {}


def _gauss_taps():
    g = np.array(
        [math.exp(-((i - WS // 2) ** 2) / (2.0 * SIGMA**2)) for i in range(WS)],
        dtype=np.float64,
    )
    g = g / g.sum()
    return g.astype(np.float32)


def _make_bands(g):
    """Banded 1-D conv matrices.

    A[h, h'] = g[h - h' + 5]   (H-conv: out[h'] = sum_h A[h,h'] x[h])
    Bm[w, w'] = g[w - w' + 5]  (W-conv)
    """
    A = np.zeros((H, H), dtype=np.float32)
    for h in range(H):
        for hp in range(max(0, h - 5), min(H, h + 6)):
            A[h, hp] = g[h - hp + 5]
    Bm = np.zeros((W, W), dtype=np.float32)
    for w in range(W):
        for wp in range(max(0, w - 5), min(W, w + 6)):
            Bm[w, wp] = g[w - wp + 5]
    bf = np.float32  # cast to bf16 happens via ml_dtypes below
    import ml_dtypes

    to_bf16 = lambda a: a.astype(ml_dtypes.bfloat16)
    return (
        to_bf16(A[0:HHI, :]),  # bandA_hi [128, 192]
        to_bf16(A[HHI:H, :]),  # bandA_lo [64, 192]
        to_bf16(Bm[0:WHALF, :]),  # bandB0  [128, 256]
        to_bf16(Bm[WHALF:W, :]),  # bandB1  [128, 256]
    )


def _build_nc():
    """Build + finalize the per-core Bass program (same program on all 8)."""
    nc = bacc.Bacc(None, target_bir_lowering=False, debug=False)

    x_in = nc.dram_tensor("img1", [PPC, H, W], F32, kind="ExternalInput")
    y_in = nc.dram_tensor("img2", [PPC, H, W], F32, kind="ExternalInput")
    bandA_hi = nc.dram_tensor("bandA_hi", [HHI, H], BF16, kind="ExternalInput")
    bandA_lo = nc.dram_tensor("bandA_lo", [HLO, H], BF16, kind="ExternalInput")
    bandB0 = nc.dram_tensor("bandB0", [WHALF, W], BF16, kind="ExternalInput")
    bandB1 = nc.dram_tensor("bandB1", [WHALF, W], BF16, kind="ExternalInput")
    partials_out = nc.dram_tensor("partials", [128, 1], F32, kind="ExternalOutput")

    SQH = math.sqrt(0.5)
    DG = 4  # pairs per DMA group

    with tile.TileContext(nc) as tc:
        with (
            tc.tile_pool(name="consts", bufs=1) as consts,
            tc.tile_pool(name="inp", bufs=3) as inp,
            tc.tile_pool(name="planes", bufs=3) as planes_pool,
            tc.tile_pool(name="zt", bufs=3) as zt_pool,
            tc.tile_pool(name="stg", bufs=3) as stg,
            tc.tile_pool(name="pw", bufs=2) as pw,
            tc.tile_pool(name="acc", bufs=1) as accp,
            tc.tile_pool(name="hpsum", bufs=2, space="PSUM") as hpsum,
            tc.tile_pool(name="wpsum", bufs=2, space="PSUM") as wpsum,
        ):
            # Constants into SBUF.
            A_hi = consts.tile([HHI, H], BF16)
            nc.gpsimd.dma_start(out=A_hi, in_=bandA_hi[:, :])
            A_lo = consts.tile([HLO, H], BF16)
            nc.gpsimd.dma_start(out=A_lo, in_=bandA_lo[:, :])
            B0 = consts.tile([WHALF, W], BF16)
            nc.gpsimd.dma_start(out=B0, in_=bandB0[:, :])
            B1 = consts.tile([WHALF, W], BF16)
            nc.gpsimd.dma_start(out=B1, in_=bandB1[:, :])
            # Rank-1 constant fold: ones[K=1,M=128] x crow[K=1,N=384] adds
            # per-column constants to a PSUM tile (2*(C1+C2) on the A2 half,
            # pre-ACT-scale 0.5).
            onesc = consts.tile([1, 128], BF16)
            nc.vector.memset(onesc, 1.0)
            crow = consts.tile([1, 2 * H], BF16)
            nc.vector.memset(crow[:, 0:H], 2.0 * (C1 + C2))
            nc.vector.memset(crow[:, H : 2 * H], 0.0)

            accT = accp.tile([128, 2 * H], BF16)
            nc.vector.memset(accT, 0.0)
            acc32 = accp.tile([128, 4], F32)
            nc.vector.memset(acc32, 0.0)

            def load_group(pg):
                p0 = pg * DG
                xgh = inp.tile([HHI, DG, W], BF16, tag="xgh", name="xgh")
                nc.gpsimd.dma_start(
                    out=xgh, in_=x_in[p0 : p0 + DG, 0:HHI, :].transpose([1, 0, 2])
                )
                xgl = inp.tile([HLO, DG, W], BF16, tag="xgl", name="xgl")
                nc.gpsimd.dma_start(
                    out=xgl, in_=x_in[p0 : p0 + DG, HHI:H, :].transpose([1, 0, 2])
                )
                ygh = inp.tile([HHI, DG, W], BF16, tag="ygh", name="ygh")
                nc.gpsimd.dma_start(
                    out=ygh, in_=y_in[p0 : p0 + DG, 0:HHI, :].transpose([1, 0, 2])
                )
                ygl = inp.tile([HLO, DG, W], BF16, tag="ygl", name="ygl")
                nc.gpsimd.dma_start(
                    out=ygl, in_=y_in[p0 : p0 + DG, HHI:H, :].transpose([1, 0, 2])
                )
                return xgh, xgl, ygh, ygl

            def stage2(grp, pj):
                """s,d,s2,d2: hi chunks on gpsimd, lo chunks on DVE."""
                xgh, xgl, ygh, ygl = grp
                xh, xl = xgh[:, pj, :], xgl[:, pj, :]
                yh, yl = ygh[:, pj, :], ygl[:, pj, :]
                sh = planes_pool.tile([HHI, W], BF16, tag="sh", name="sh")
                nc.gpsimd.tensor_add(sh, xh, yh)
                dh = planes_pool.tile([HHI, W], BF16, tag="dh", name="dh")
                nc.gpsimd.tensor_sub(dh, xh, yh)
                sl = planes_pool.tile([HLO, W], BF16, tag="sl", name="sl")
                nc.vector.tensor_add(sl, xl, yl)
                dl = planes_pool.tile([HLO, W], BF16, tag="dl", name="dl")
                nc.vector.tensor_sub(dl, xl, yl)
                s2h = planes_pool.tile([HHI, W], BF16, tag="s2h", name="s2h")
                nc.gpsimd.tensor_mul(s2h, sh, sh)
                d2h = planes_pool.tile([HHI, W], BF16, tag="d2h", name="d2h")
                nc.gpsimd.tensor_mul(d2h, dh, dh)
                s2l = planes_pool.tile([HLO, W], BF16, tag="s2l", name="s2l")
                nc.vector.tensor_mul(s2l, sl, sl)
                d2l = planes_pool.tile([HLO, W], BF16, tag="d2l", name="d2l")
                nc.vector.tensor_mul(d2l, dl, dl)
                return [((sh, sl), (dh, dl)), ((s2h, s2l), (d2h, d2l))]

            def hconv(units):
                hps = []
                for ui, (pa, pb) in enumerate(units):
                    hp = hpsum.tile([WHALF, 2, 512], F32, tag="hp", name="hp")
                    for qi, (qh, ql) in enumerate((pa, pb)):
                        for m in range(2):
                            dst = hp[:, qi, m * H : (m + 1) * H]
                            nc.tensor.matmul(
                                dst, qh[:, m * WHALF : (m + 1) * WHALF],
                                A_hi[:, :], start=True, stop=False,
                            )
                            nc.tensor.matmul(
                                dst, ql[:, m * WHALF : (m + 1) * WHALF],
                                A_lo[:, :], start=False, stop=True,
                            )
                    hps.append(hp)
                return hps

            def evac(hps):
                zsbs = []
                for ui, hp in enumerate(hps):
                    zsb = zt_pool.tile(
                        [WHALF, 2, 2 * H], BF16, tag=f"zsb{ui}", name="zsb"
                    )
                    nc.scalar.copy(out=zsb, in_=hp[:, :, 0 : 2 * H])
                    zsbs.append(zsb)
                return zsbs

            def wconv(zsbs):
                Ms = []
                for ui, zsb in enumerate(zsbs):
                    M = wpsum.tile([WHALF, 2, 512], F32, tag="M", name="M")
                    for m in range(2):
                        dst = M[:, m, 0 : 2 * H]
                        if m == 0:
                            nc.tensor.matmul(
                                dst, B0[:, 0:WHALF], zsb[:, :, 0:H],
                                start=True, stop=False,
                            )
                            nc.tensor.matmul(
                                dst, B1[0:5, 0:WHALF], zsb[0:5, :, H : 2 * H],
                                start=False, stop=(ui == 0),
                            )
                        else:
                            nc.tensor.matmul(
                                dst, B0[64:WHALF, WHALF:W],
                                zsb[64:WHALF, :, 0:H],
                                start=True, stop=False,
                            )
                            nc.tensor.matmul(
                                dst, B1[:, WHALF:W], zsb[:, :, H : 2 * H],
                                start=False, stop=(ui == 0),
                            )
                        if ui == 1:
                            # fold per-column constants into [A2|B2]
                            nc.tensor.matmul(
                                dst, onesc[:, :], crow[:, :],
                                start=False, stop=True,
                            )
                    Ms.append(M)
                return Ms

            def entry(Ms):
                ent = []
                for ui, M in enumerate(Ms):
                    t = stg.tile(
                        [WHALF, 2, 2 * H], BF16, tag=f"ent{ui}", name=f"ent{ui}"
                    )
                    if ui == 0:
                        nc.scalar.activation(
                            out=t, in_=M[:, :, 0 : 2 * H],
                            func=mybir.ActivationFunctionType.Square, scale=SQH,
                        )
                    else:
                        nc.scalar.activation(
                            out=t, in_=M[:, :, 0 : 2 * H],
                            func=mybir.ActivationFunctionType.Copy, scale=0.5,
                        )
                    ent.append(t)
                return ent

            def pointwise(ent):
                SQt, ABt = ent
                Ssq = SQt[:, :, 0:H]
                Qsq = SQt[:, :, H : 2 * H]
                As = ABt[:, :, 0:H]   # A2/2 + C1 + C2
                Bs = ABt[:, :, H : 2 * H]  # B2/2

                def pwt(tag, dt=BF16):
                    return pw.tile([WHALF, 2, H], dt, tag=tag, name=tag)

                F = pwt("F")
                nc.vector.tensor_sub(F, Ssq, Qsq)  # 2P
                G = pwt("G")
                nc.vector.tensor_add(G, Ssq, Qsq)  # d1
                Am = pwt("Am")
                nc.vector.tensor_sub(Am, As, Bs)  # 2D + C1 + C2
                Ap = pwt("Ap")
                nc.vector.tensor_add(Ap, As, Bs)  # U + C1 + C2
                num1 = pwt("num1")
                nc.vector.tensor_scalar(
                    out=num1, in0=F, scalar1=C1, scalar2=None,
                    op0=mybir.AluOpType.add,
                )
                den1 = pwt("den1")
                nc.vector.tensor_scalar(
                    out=den1, in0=G, scalar1=C1, scalar2=None,
                    op0=mybir.AluOpType.add,
                )
                num2 = pwt("num2")
                nc.vector.tensor_sub(num2, Am, num1)  # 2D + C2 - 2P
                den2 = pwt("den2")
                nc.vector.tensor_sub(den2, Ap, den1)  # U + C2 - d1
                num = pwt("num")
                nc.vector.tensor_mul(num, num1, num2)
                den = pwt("den", F32)
                nc.vector.tensor_mul(den, den1, den2)
                r = pwt("r", F32)
                nc.vector.reciprocal_approx_fast(out=r, in_=den)
                sp = pwt("sp")
                nc.vector.tensor_mul(sp, num, r)
                nc.vector.tensor_add(
                    accT, accT, sp.rearrange("p a h -> p (a h)")
                )

            # Software pipeline with per-tick emission order tuned for the
            # in-order engine streams (PE must never wait on the DVE tail).
            grp = load_group(0)
            units = stage2(grp, 0)
            hps = hconv(units)
            pending = evac(hps)
            for p in range(1, PPC + 1):
                if p < PPC:
                    if p % DG == 0:
                        grp = load_group(p // DG)
                    units = stage2(grp, p % DG)
                    hps = hconv(units)
                Ms = wconv(pending)
                ent = entry(Ms)
                if p < PPC:
                    pending = evac(hps)
                pointwise(ent)

            # ---- final reduction: bf16 accT -> f32 scalar-ish ----
            nc.vector.tensor_reduce(
                acc32[:, 0:1], accT, axis=mybir.AxisListType.X,
                op=mybir.AluOpType.add,
            )
            nc.gpsimd.dma_start(out=partials_out[:, :], in_=acc32[:, 0:1])

    nc.finalize()
    return nc


def _get_nc():
    if "nc" not in _CACHE:
        _CACHE["nc"] = _build_nc()
    return _CACHE["nc"]


def _host_kl(img1, img2):
    """Host-side KLDiv branch value (only consumed when ssim > 0.75)."""
    x1 = img1.reshape(B, H * W).astype(np.float32)
    x2 = img2.reshape(B, H * W).astype(np.float32)

    def row_hist(x):
        mn = x.min(axis=1, keepdims=True)
        mx = x.max(axis=1, keepdims=True)
        width = mx - mn
        scaled = np.where(width > 0, (x - mn) * NBIN / width, 0.0)
        idx = np.clip(scaled.astype(np.int32), 0, NBIN - 1)
        h = np.zeros((B, NBIN), np.float32)
        for r in range(B):
            h[r] = np.bincount(idx[r], minlength=NBIN)
        return h

    def softmax(h):
        e = np.exp(h - h.max(axis=1, keepdims=True))
        return e / e.sum(axis=1, keepdims=True)

    p1 = softmax(row_hist(x1))
    p2 = softmax(row_hist(x2))
    return float(np.sum(np.exp(p2) * (p2 - p1)) / B)


def kernel(img1, img2, window):
    img1 = np.asarray(img1, dtype=np.float32)
    img2 = np.asarray(img2, dtype=np.float32)
    window = np.asarray(window, dtype=np.float32)

    # Recover the 1-D taps from the passed 2-D window (rows sum to g_i since
    # sum(g)=1), keeping the kernel faithful to the provided window input.
    g = window[0, 0].sum(axis=1)
    g = (g / g.sum()).astype(np.float32)
    bandA_hi, bandA_lo, bandB0, bandB1 = _make_bands(g)

    x = img1.reshape(B, H, W)
    y = img2.reshape(B, H, W)

    nc = _get_nc()
    in_maps = []
    for c in range(NCORES):
        sl = slice(c * PPC, (c + 1) * PPC)
        in_maps.append(
            {
                "img1": np.ascontiguousarray(x[sl]),
                "img2": np.ascontiguousarray(y[sl]),
                "bandA_hi": bandA_hi,
                "bandA_lo": bandA_lo,
                "bandB0": bandB0,
                "bandB1": bandB1,
            }
        )

    res = run_bass_kernel_spmd(nc, in_maps, core_ids=list(range(NCORES)))
    total = 0.0
    for c in range(NCORES):
        total += float(res.results[c]["partials"].sum())
    ssim = total / float(B * C * H * W)

    if ssim > 0.75:
        out = _host_kl(img1, img2) + 1.0 - ssim
    else:
        out = 1.0 - ssim
    return np.float32(out)


if __name__ == "__main__":
    rng = np.random.default_rng(0)
    i1 = rng.standard_normal((B, C, H, W), dtype=np.float32)
    i2 = rng.standard_normal((B, C, H, W), dtype=np.float32)
    g = _gauss_taps()
    w2 = np.outer(g, g).astype(np.float32)[None, None]
    print("out:", kernel(i1, i2, w2))

